# revision 26
# baseline (speedup 1.0000x reference)
"""Trainium2 Bass kernel for nn_NestedConv (gnn_message_passing).

Math (per b, i):
    Xm       = X[b,i] * mask[b,i,:,None]                 # (N,D), rows k masked
    h1       = relu(Xm @ W1 + b1)                        # (N,D)
    h        = relu(h1 @ W2 + b2)                        # (N,D)
    out[b,i] = (A[b].T @ h) * mask[b,i,:,None]           # (N,D), rows j masked

Key restructuring vs the obvious dataflow:
  - X is uploaded host-pre-transposed+bf16 as XT[b, (half,d), (g,p,k)] so the
    MLP contraction dim d sits on partitions with no on-chip transposes.
  - The input row-mask is deferred: rowwise MLP maps 0-rows to 0-rows when
    b1=b2=0, so masking h (natural layout) == masking X. For nonzero biases
    the exact correction  out += (A^T (1-m_i)) c^T  with c = MLP(0-row) is
    added per batch (one extra matmul) + per group (two DVE ops).
  - All matmuls bf16 (A and mask are 0/1-exact in bf16):
      mm1: stationary blockdiag[W1,W1], moving XT slice      -> h1^T pairs
      mm2: stationary h1^T pair, moving blockdiag[W2,W2]     -> h natural
      mm3: stationary A[b], moving h (8 i's batched)         -> out rows j
  - Work is grouped in super-groups of 16 root nodes (1024-wide tiles) to
    amortize fixed op costs; mm2's psH and mm3's psO reuse the same 2-bank
    PSUM tile sequentially so everything fits in 8 banks double-buffered.
  - Elementwise work is split across ACT/DVE/GPSIMD:
      relu1 on ACT; relu2+hmask alternates (even sg: ACT relu + GPSIMD
      mask-mul; odd sg: one fused DVE (max,mult) op); outmask on DVE.
  - Input loaded in 512 KiB chunks (compute starts ~2 us in), output stored
    bf16 from a per-batch SBUF buffer in 1 MiB chunks; host re-transposes.

Sharding: data-parallel over batch dim B=64 across 8 NeuronCores (8 b's each).
"""

import sys

sys.path.insert(0, "/opt/trn_rl_repo")

import numpy as np

B, N, D = 64, 128, 64
NC = 8
BSH = B // NC  # batches per core
G = 8  # root nodes i per group
NG = N // G  # groups per batch
GF = G * D  # free size of one group: 512
NP = G // 2  # stationary pairs per group: 4

_built = {}
_last_in_maps = None


def _build(bias_mode: bool, cfg: dict, bsh: int = BSH, ng: int = NG):
    import concourse.bacc as bacc
    import concourse.mybir as mybir
    from concourse import tile
    from concourse.bass_interp import get_hw_module

    f32 = mybir.dt.float32
    bf16 = mybir.dt.bfloat16
    Relu = mybir.ActivationFunctionType.Relu
    Alu = mybir.AluOpType

    relu2_mode = cfg.get("relu2", "alt")  # alt | dve | act_gps
    # global group indices (mod 16) that use the fused-DVE relu2 path;
    # 7/16 balances ACT (relu1 + the other 9/16 relu2) against DVE
    # (outmask + these). The last batch goes 1/2 so the slow GPSIMD
    # mask-mul stays off the drain critical path.
    dve_pat = cfg.get("dve_pat", (1, 3, 5, 7, 9, 11, 13))
    sgn = 1 if bias_mode else cfg.get("sgn", 2)  # groups sharing one mm1/relu1
    ocst = cfg.get("ochunk", 4)  # groups per output-store chunk
    icn = cfg.get("ichunk", 4)  # input chunks per batch

    nc = bacc.Bacc("TRN2", target_bir_lowering=False, debug=False, num_devices=1)

    XT_d = nc.dram_tensor("XT", [bsh, 128, ng * GF], bf16, kind="ExternalInput").ap()
    A_d = nc.dram_tensor("A", [bsh, 128, 128], bf16, kind="ExternalInput").ap()
    MT_d = nc.dram_tensor("MT", [bsh, 128, 128], f32, kind="ExternalInput").ap()
    MTB_d = nc.dram_tensor("MTB", [bsh, 128, 128], bf16, kind="ExternalInput").ap()
    W1_d = nc.dram_tensor("W1Q", [128, 128], bf16, kind="ExternalInput").ap()
    W2_d = nc.dram_tensor("W2Q", [128, 128], bf16, kind="ExternalInput").ap()
    B1_d = nc.dram_tensor("B1D", [128, 1], f32, kind="ExternalInput").ap()
    if bias_mode:
        B2_d = nc.dram_tensor("B2BC", [128, GF], f32, kind="ExternalInput").ap()
        CB_d = nc.dram_tensor("CB", [128, GF], f32, kind="ExternalInput").ap()
    O_d = nc.dram_tensor("OUT", [bsh, 128, ng * GF], bf16, kind="ExternalOutput").ap()

    with tile.TileContext(nc) as tc:
        with (
            tc.tile_pool(name="const", bufs=1) as cpool,
            tc.tile_pool(name="xb", bufs=2) as xbpool,
            tc.tile_pool(name="bmeta", bufs=2) as bmpool,
            tc.tile_pool(name="ob", bufs=2) as obpool,
            tc.tile_pool(name="h1", bufs=3) as h1pool,
            tc.tile_pool(name="ht", bufs=3) as htpool,
            tc.tile_pool(name="ht0", bufs=2) as ht0pool,
            tc.tile_pool(name="psH1", bufs=2, space="PSUM") as psH1pool,
            tc.tile_pool(name="psH", bufs=2, space="PSUM") as psHpool,
            tc.tile_pool(name="psO", bufs=2, space="PSUM") as psOpool,
        ):
            ub_pool = tmpc_pool = psU_pool = None
            if bias_mode:
                ub_pool = tc.tile_pool(name="ub", bufs=2).__enter__()
                tmpc_pool = tc.tile_pool(name="tmpc", bufs=2).__enter__()
                psU_pool = tc.tile_pool(name="psU", bufs=1, space="PSUM").__enter__()

            w1q = cpool.tile([128, 128], bf16, tag="w1q")
            nc.sync.dma_start(w1q[:, :], W1_d)
            w2q = cpool.tile([128, 128], bf16, tag="w2q")
            nc.sync.dma_start(w2q[:, :], W2_d)
            b1d = cpool.tile([128, 1], f32, tag="b1d")
            nc.sync.dma_start(b1d[:, :], B1_d)
            # Warm the ACT function table during the first input DMA so the
            # one-time ACT_TABLE_LOAD (~1.3us) is off the critical path.
            warm = cpool.tile([128, 1], f32, tag="warm")
            nc.scalar.activation(warm[:, :], b1d[:, :], Relu)
            if bias_mode:
                b2bc = cpool.tile([128, GF], f32, tag="b2bc")
                nc.sync.dma_start(b2bc[:, :], B2_d)
                cb = cpool.tile([128, GF], f32, tag="cb")
                nc.sync.dma_start(cb[:, :], CB_d)

            batch_tiles = {}
            FW = ng * GF  # full batch free width: 8192

            def load_batch(b):
                if b >= bsh:
                    return
                xbT = xbpool.tile([128, FW], bf16)
                cw = FW // icn
                # batch 0: halve the first chunk so compute starts sooner
                cuts = [0, cw // 2, cw] if b == 0 else [0, cw]
                cuts += [c * cw for c in range(2, icn + 1)]
                nc.sync.dma_start(
                    xbT[:, cuts[0] : cuts[1]], XT_d[b][:, cuts[0] : cuts[1]]
                )
                at = bmpool.tile([128, 128], bf16, tag="at")
                nc.sync.dma_start(at[:, :], A_d[b])
                mt = bmpool.tile([128, 128], f32, tag="mt")
                nc.sync.dma_start(mt[:, :], MT_d[b])
                mtb = bmpool.tile([128, 128], bf16, tag="mtb")
                nc.sync.dma_start(mtb[:, :], MTB_d[b])
                for c0, c1 in zip(cuts[1:], cuts[2:]):
                    nc.sync.dma_start(xbT[:, c0:c1], XT_d[b][:, c0:c1])
                batch_tiles[b] = dict(xbT=xbT, at=at, mt=mt, mtb=mtb)

            load_batch(0)

            total = bsh * ng
            ctxs = [None] * total

            def S0(i):
                # one S0 covers a super-group of sgn consecutive groups:
                # sgn N=512 matmuls into one PSUM tile, a single wide
                # relu1, shared h1t tile.
                b, g = divmod(i, ng)
                if g == ng // 4:
                    load_batch(b + 1)
                t = batch_tiles[b]
                if g == 0:
                    if bias_mode:
                        # U[j,i] = sum_k A[k,j] (1 - m[k,i]); c-column correction
                        omtb = ht0pool.tile([128, 128], bf16, tag="omtb")
                        nc.vector.tensor_scalar(
                            omtb[:, :], t["mt"][:, :], 1.0, -1.0,
                            Alu.subtract, Alu.mult,
                        )
                        psU = psU_pool.tile([128, 128], f32)
                        nc.tensor.matmul(
                            psU[:, :], t["at"][:, :], omtb[:, :],
                            start=True, stop=True,
                        )
                        ub = ub_pool.tile([128, 128], f32)
                        nc.scalar.copy(ub[:, :], psU[:, :])
                        batch_tiles[b]["ub"] = ub
                    obuf = obpool.tile([128, FW], bf16)
                    batch_tiles[b]["obuf"] = obuf

                psH1 = psH1pool.tile([128, sgn * GF], f32)
                for s in range(sgn):
                    gs = g + s
                    nc.tensor.matmul(
                        psH1[:, s * GF : (s + 1) * GF],
                        w1q[:, :],
                        t["xbT"][:, gs * GF : (gs + 1) * GF],
                        start=True, stop=True,
                    )
                h1t = h1pool.tile([128, sgn * GF], bf16)
                nc.scalar.activation(h1t[:, :], psH1[:, :], Relu, bias=b1d[:, 0:1])
                for s in range(sgn):
                    ctx = dict(b=b, g=g + s, **t)
                    ctx["obuf"] = batch_tiles[b]["obuf"]
                    if bias_mode:
                        ctx["ub"] = batch_tiles[b]["ub"]
                    ctx["h1t"] = h1t
                    ctx["h1off"] = s * GF
                    ctxs[i + s] = ctx

            def S1(i):
                ctx = ctxs[i]
                g = ctx["g"]
                i0 = g * G
                h1t = ctx["h1t"]
                off = ctx["h1off"]
                psH = psHpool.tile([128, GF], f32)
                for p in range(NP):
                    nc.tensor.matmul(
                        psH[:, p * 128 : (p + 1) * 128],
                        h1t[:, off + p * 128 : off + (p + 1) * 128],
                        w2q[:, :],
                        start=True, stop=True,
                    )
                if bias_mode:
                    nc.vector.tensor_add(psH[:, :], psH[:, :], b2bc[:, :])
                mtg = ctx["mt"][:, i0 : i0 + G].unsqueeze(2).broadcast_to([128, G, D])
                ht = htpool.tile([128, GF], bf16)
                ht3 = ht[:, :].rearrange("k (i d) -> k i d", i=G)
                psH3 = psH[:, :].rearrange("k (i d) -> k i d", i=G)
                if ctx["b"] == bsh - 1:
                    use_dve = g % 2 == 1
                else:
                    use_dve = (ctx["b"] * ng + g) % 16 in dve_pat
                use_dve = relu2_mode == "dve" or (
                    relu2_mode == "alt" and use_dve
                )
                if use_dve:
                    # ht = relu(psH) * m  ==  (psH max 0) * m, one fused DVE op
                    nc.vector.scalar_tensor_tensor(
                        ht3, psH3, 0.0, mtg, Alu.max, Alu.mult
                    )
                else:
                    ht0 = ht0pool.tile([128, GF], bf16, tag="ht0")
                    nc.scalar.activation(ht0[:, :], psH[:, :], Relu)
                    mtgb = (
                        ctx["mtb"][:, i0 : i0 + G]
                        .unsqueeze(2)
                        .broadcast_to([128, G, D])
                    )
                    nc.gpsimd.tensor_mul(
                        ht3, ht0[:, :].rearrange("k (i d) -> k i d", i=G), mtgb
                    )
                ctx["ht"] = ht

            def S2(i):
                ctx = ctxs[i]
                b, g = ctx["b"], ctx["g"]
                i0 = g * G
                psO = psOpool.tile([128, GF], f32)
                nc.tensor.matmul(
                    psO[:, :], ctx["at"][:, :], ctx["ht"][:, :],
                    start=True, stop=True,
                )
                psO3 = psO[:, :].rearrange("j (i d) -> j i d", i=G)
                mtg = ctx["mt"][:, i0 : i0 + G].unsqueeze(2).broadcast_to([128, G, D])
                if bias_mode:
                    tmpc = tmpc_pool.tile([128, GF], f32)
                    ubg = (
                        ctx["ub"][:, i0 : i0 + G]
                        .unsqueeze(2)
                        .broadcast_to([128, G, D])
                    )
                    nc.vector.tensor_mul(
                        tmpc[:, :].rearrange("j (i d) -> j i d", i=G),
                        ubg,
                        cb[:, :].rearrange("j (i d) -> j i d", i=G),
                    )
                    nc.vector.tensor_add(psO[:, :], psO[:, :], tmpc[:, :])
                ot3 = (
                    ctx["obuf"][:, g * GF : (g + 1) * GF]
                    .rearrange("j (i d) -> j i d", i=G)
                )
                nc.vector.tensor_mul(ot3, psO3, mtg)
                if (g + 1) % ocst == 0:
                    c0 = (g + 1 - ocst) * GF
                    c1 = (g + 1) * GF
                    # output stores can ride the SWDGE ring (gpsimd) so they
                    # never queue ahead of input prefetches on the HWDGE ring
                    oeng = nc.gpsimd if cfg.get("oswdge") else nc.sync
                    oeng.dma_start(O_d[b][:, c0:c1], ctx["obuf"][:, c0:c1])

            for i in range(total):
                if i % sgn == 0:
                    S0(i)
                if i >= 1:
                    S1(i - 1)
                if i >= 2:
                    S2(i - 2)
            S1(total - 1)
            S2(total - 2)
            S2(total - 1)

            if bias_mode:
                ub_pool.__exit__(None, None, None)
                tmpc_pool.__exit__(None, None, None)
                psU_pool.__exit__(None, None, None)

    nc.compile()
    nc.m = get_hw_module(nc.m)
    return nc


def kernel(X, A, mask, W1, b1, W2, b2):
    import ml_dtypes
    from concourse.bass_utils import run_bass_kernel_spmd

    bf = ml_dtypes.bfloat16
    cfg = dict(
        relu2="alt", dve_pat=(1, 3, 5, 7, 9, 11, 13), sgn=2, ochunk=4, ichunk=4
    )
    import json
    import os

    if os.environ.get("KCFG"):
        _o = json.loads(os.environ["KCFG"])
        if "dve_pat" in _o:
            _o["dve_pat"] = tuple(_o["dve_pat"])
        cfg.update(_o)

    X = np.asarray(X, dtype=np.float32)
    A = np.asarray(A, dtype=np.float32)
    mask = np.asarray(mask)
    W1 = np.asarray(W1, dtype=np.float32)
    W2 = np.asarray(W2, dtype=np.float32)
    b1 = np.asarray(b1, dtype=np.float32)
    b2 = np.asarray(b2, dtype=np.float32)

    bias_mode = bool(np.any(b1 != 0.0) or np.any(b2 != 0.0))
    key = (bias_mode, tuple(sorted(cfg.items())))
    if key not in _built:
        _built[key] = _build(bias_mode, cfg)
    nc = _built[key]

    # XT[b, (half,d), (g,p,k)] for i = 8g + 2p + half
    XT = np.ascontiguousarray(
        X.reshape(B, NG, NP, 2, N, D).transpose(0, 3, 5, 1, 2, 4)
    ).reshape(B, 128, NG * GF).astype(bf)
    Ab = A.astype(bf)
    MTf = np.ascontiguousarray(np.swapaxes(mask, 1, 2)).astype(np.float32)
    MTb = MTf.astype(bf)

    w1q = np.zeros((128, 128), dtype=np.float32)
    w1q[0:64, 0:64] = W1
    w1q[64:128, 64:128] = W1
    w2q = np.zeros((128, 128), dtype=np.float32)
    w2q[0:64, 0:64] = W2
    w2q[64:128, 64:128] = W2
    b1d = np.concatenate([b1, b1], axis=0).reshape(128, 1).astype(np.float32)

    shared = {
        "W1Q": w1q.astype(bf),
        "W2Q": w2q.astype(bf),
        "B1D": b1d,
    }
    if bias_mode:
        c = np.maximum(np.maximum(b1, 0.0) @ W2 + b2, 0.0).astype(np.float32)
        shared["B2BC"] = np.tile(b2, (128, G)).astype(np.float32)
        shared["CB"] = np.tile(c, (128, G)).astype(np.float32)

    in_maps = []
    for cid in range(NC):
        sl = slice(cid * BSH, (cid + 1) * BSH)
        in_maps.append(
            {"XT": XT[sl], "A": Ab[sl], "MT": MTf[sl], "MTB": MTb[sl], **shared}
        )
    global _last_in_maps
    _last_in_maps = in_maps

    try:
        res = run_bass_kernel_spmd(nc, in_maps, core_ids=list(range(NC)))
    except Exception:
        res = run_bass_kernel_spmd(nc, in_maps, core_ids=list(range(NC)))
    OT = np.concatenate([res.results[c]["OUT"] for c in range(NC)], axis=0)
    # OT[b, j, (g, ig, d)] -> out[b, i=8g+ig, j, d]
    out = (
        OT.astype(np.float32)
        .reshape(B, N, NG, G, D)
        .transpose(0, 2, 3, 1, 4)
        .reshape(B, N, N, D)
    )
    return np.ascontiguousarray(out)


# revision 31
# speedup vs baseline: 1.0731x; 1.0731x over previous
"""Trainium2 Bass kernel for nn_NestedConv (gnn_message_passing).

Math (per b, i):
    Xm       = X[b,i] * mask[b,i,:,None]                 # (N,D), rows k masked
    h1       = relu(Xm @ W1 + b1)                        # (N,D)
    h        = relu(h1 @ W2 + b2)                        # (N,D)
    out[b,i] = (A[b].T @ h) * mask[b,i,:,None]           # (N,D), rows j masked

Key restructuring vs the obvious dataflow:
  - X is uploaded host-pre-transposed+bf16 as XT[b, (half,d), (g,p,k)] so the
    MLP contraction dim d sits on partitions with no on-chip transposes.
  - The input row-mask is deferred: rowwise MLP maps 0-rows to 0-rows when
    b1=b2=0, so masking h (natural layout) == masking X. For nonzero biases
    the exact correction  out += (A^T (1-m_i)) c^T  with c = MLP(0-row) is
    added per batch (one extra matmul) + per group (two DVE ops).
  - All matmuls bf16 (A and mask are 0/1-exact in bf16):
      mm1: stationary blockdiag[W1,W1], moving XT slice      -> h1^T pairs
      mm2: stationary h1^T pair, moving blockdiag[W2,W2]     -> h natural
      mm3: stationary A[b], moving h (8 i's batched)         -> out rows j
  - Work is grouped in super-groups of 16 root nodes (1024-wide tiles) to
    amortize fixed op costs; mm2's psH and mm3's psO reuse the same 2-bank
    PSUM tile sequentially so everything fits in 8 banks double-buffered.
  - Elementwise work is split across ACT/DVE/GPSIMD:
      relu1 on ACT; relu2+hmask alternates (even sg: ACT relu + GPSIMD
      mask-mul; odd sg: one fused DVE (max,mult) op); outmask on DVE.
  - Input loaded in 512 KiB chunks (compute starts ~2 us in), output stored
    bf16 from a per-batch SBUF buffer in 1 MiB chunks; host re-transposes.

Sharding: data-parallel over batch dim B=64 across 8 NeuronCores (8 b's each).
"""

import sys

sys.path.insert(0, "/opt/trn_rl_repo")

import numpy as np

B, N, D = 64, 128, 64
NC = 8
BSH = B // NC  # batches per core
G = 8  # root nodes i per group
NG = N // G  # groups per batch
GF = G * D  # free size of one group: 512
NP = G // 2  # stationary pairs per group: 4

_built = {}
_last_in_maps = None


def _build(bias_mode: bool, cfg: dict, bsh: int = BSH, ng: int = NG):
    import concourse.bacc as bacc
    import concourse.mybir as mybir
    from concourse import tile
    from concourse.bass_interp import get_hw_module

    f32 = mybir.dt.float32
    bf16 = mybir.dt.bfloat16
    Relu = mybir.ActivationFunctionType.Relu
    Alu = mybir.AluOpType

    relu2_mode = cfg.get("relu2", "alt")  # alt | dve | act_gps
    # global group indices (mod 16) that use the fused-DVE relu2 path;
    # 7/16 balances ACT (relu1 + the other 9/16 relu2) against DVE
    # (outmask + these). The last batch goes 1/2 so the slow GPSIMD
    # mask-mul stays off the drain critical path.
    dve_pat = cfg.get("dve_pat", (1, 3, 5, 7, 9, 11, 13))
    sgn = 1 if bias_mode else cfg.get("sgn", 2)  # groups sharing one mm1/relu1
    ocst = cfg.get("ochunk", 4)  # groups per output-store chunk
    icn = cfg.get("ichunk", 4)  # input chunks per batch

    nc = bacc.Bacc("TRN2", target_bir_lowering=False, debug=False, num_devices=1)

    XT_d = nc.dram_tensor("XT", [bsh, 128, ng * GF], bf16, kind="ExternalInput").ap()
    A_d = nc.dram_tensor("A", [bsh, 128, 128], bf16, kind="ExternalInput").ap()
    MT_d = nc.dram_tensor("MT", [bsh, 128, 128], f32, kind="ExternalInput").ap()
    MTB_d = nc.dram_tensor("MTB", [bsh, 128, 128], bf16, kind="ExternalInput").ap()
    W1_d = nc.dram_tensor("W1Q", [128, 128], bf16, kind="ExternalInput").ap()
    W2_d = nc.dram_tensor("W2Q", [128, 128], bf16, kind="ExternalInput").ap()
    B1_d = nc.dram_tensor("B1D", [128, 1], f32, kind="ExternalInput").ap()
    if bias_mode:
        B2_d = nc.dram_tensor("B2BC", [128, GF], f32, kind="ExternalInput").ap()
        CB_d = nc.dram_tensor("CB", [128, GF], f32, kind="ExternalInput").ap()
    O_d = nc.dram_tensor("OUT", [bsh, 128, ng * GF], bf16, kind="ExternalOutput").ap()

    with tile.TileContext(nc) as tc:
        with (
            tc.tile_pool(name="const", bufs=1) as cpool,
            tc.tile_pool(name="xb", bufs=2) as xbpool,
            tc.tile_pool(name="bmeta", bufs=2) as bmpool,
            tc.tile_pool(name="ob", bufs=2) as obpool,
            tc.tile_pool(name="h1", bufs=3) as h1pool,
            tc.tile_pool(name="ht", bufs=3) as htpool,
            tc.tile_pool(name="ht0", bufs=2) as ht0pool,
            tc.tile_pool(name="psH1", bufs=2, space="PSUM") as psH1pool,
            tc.tile_pool(name="psH", bufs=2, space="PSUM") as psHpool,
            tc.tile_pool(name="psO", bufs=2, space="PSUM") as psOpool,
        ):
            ub_pool = tmpc_pool = psU_pool = None
            if bias_mode:
                ub_pool = tc.tile_pool(name="ub", bufs=2).__enter__()
                tmpc_pool = tc.tile_pool(name="tmpc", bufs=2).__enter__()
                psU_pool = tc.tile_pool(name="psU", bufs=1, space="PSUM").__enter__()

            # First compute needs xbT chunk0 + w1q; issue chunk0 first (each
            # dma_start costs ~650ns of SP-queue serial issue time).
            xbT0 = xbpool.tile([128, ng * GF], bf16)
            nc.sync.dma_start(xbT0[:, 0:GF], XT_d[0][:, 0:GF])
            w1q = cpool.tile([128, 128], bf16, tag="w1q")
            nc.sync.dma_start(w1q[:, :], W1_d)
            w2q = cpool.tile([128, 128], bf16, tag="w2q")
            nc.sync.dma_start(w2q[:, :], W2_d)
            b1d = cpool.tile([128, 1], f32, tag="b1d")
            nc.sync.dma_start(b1d[:, :], B1_d)
            # Warm the ACT function table during the first input DMA so the
            # one-time ACT_TABLE_LOAD (~1.3us) is off the critical path.
            warm = cpool.tile([128, 1], f32, tag="warm")
            nc.scalar.activation(warm[:, :], b1d[:, :], Relu)
            if bias_mode:
                b2bc = cpool.tile([128, GF], f32, tag="b2bc")
                nc.sync.dma_start(b2bc[:, :], B2_d)
                cb = cpool.tile([128, GF], f32, tag="cb")
                nc.sync.dma_start(cb[:, :], CB_d)

            batch_tiles = {}
            FW = ng * GF  # full batch free width: 8192

            def load_batch(b):
                if b >= bsh:
                    return
                cw = FW // icn
                if b == 0:
                    # chunk0 (one group) was issued before the consts
                    xbT = xbT0
                    cuts = [GF, cw] + [c * cw for c in range(2, icn + 1)]
                else:
                    xbT = xbpool.tile([128, FW], bf16)
                    cuts = [c * cw for c in range(icn + 1)]
                nc.sync.dma_start(
                    xbT[:, cuts[0] : cuts[1]], XT_d[b][:, cuts[0] : cuts[1]]
                )
                at = bmpool.tile([128, 128], bf16, tag="at")
                nc.sync.dma_start(at[:, :], A_d[b])
                mt = bmpool.tile([128, 128], f32, tag="mt")
                nc.sync.dma_start(mt[:, :], MT_d[b])
                mtb = bmpool.tile([128, 128], bf16, tag="mtb")
                nc.sync.dma_start(mtb[:, :], MTB_d[b])
                for c0, c1 in zip(cuts[1:], cuts[2:]):
                    nc.sync.dma_start(xbT[:, c0:c1], XT_d[b][:, c0:c1])
                batch_tiles[b] = dict(xbT=xbT, at=at, mt=mt, mtb=mtb)

            load_batch(0)

            total = bsh * ng
            ctxs = [None] * total

            def S0(i):
                # one S0 covers a super-group of sgn consecutive groups:
                # sgn N=512 matmuls into one PSUM tile, a single wide
                # relu1, shared h1t tile.
                b, g = divmod(i, ng)
                if g == ng // 4:
                    load_batch(b + 1)
                t = batch_tiles[b]
                if g == 0:
                    if bias_mode:
                        # U[j,i] = sum_k A[k,j] (1 - m[k,i]); c-column correction
                        omtb = ht0pool.tile([128, 128], bf16, tag="omtb")
                        nc.vector.tensor_scalar(
                            omtb[:, :], t["mt"][:, :], 1.0, -1.0,
                            Alu.subtract, Alu.mult,
                        )
                        psU = psU_pool.tile([128, 128], f32)
                        nc.tensor.matmul(
                            psU[:, :], t["at"][:, :], omtb[:, :],
                            start=True, stop=True,
                        )
                        ub = ub_pool.tile([128, 128], f32)
                        nc.scalar.copy(ub[:, :], psU[:, :])
                        batch_tiles[b]["ub"] = ub
                    obuf = obpool.tile([128, FW], bf16)
                    batch_tiles[b]["obuf"] = obuf

                psH1 = psH1pool.tile([128, sgn * GF], f32)
                for s in range(sgn):
                    gs = g + s
                    nc.tensor.matmul(
                        psH1[:, s * GF : (s + 1) * GF],
                        w1q[:, :],
                        t["xbT"][:, gs * GF : (gs + 1) * GF],
                        start=True, stop=True,
                    )
                h1t = h1pool.tile([128, sgn * GF], bf16)
                nc.scalar.activation(h1t[:, :], psH1[:, :], Relu, bias=b1d[:, 0:1])
                for s in range(sgn):
                    ctx = dict(b=b, g=g + s, **t)
                    ctx["obuf"] = batch_tiles[b]["obuf"]
                    if bias_mode:
                        ctx["ub"] = batch_tiles[b]["ub"]
                    ctx["h1t"] = h1t
                    ctx["h1off"] = s * GF
                    ctxs[i + s] = ctx

            def S1(i):
                ctx = ctxs[i]
                g = ctx["g"]
                i0 = g * G
                h1t = ctx["h1t"]
                off = ctx["h1off"]
                psH = psHpool.tile([128, GF], f32)
                for p in range(NP):
                    nc.tensor.matmul(
                        psH[:, p * 128 : (p + 1) * 128],
                        h1t[:, off + p * 128 : off + (p + 1) * 128],
                        w2q[:, :],
                        start=True, stop=True,
                    )
                if bias_mode:
                    nc.vector.tensor_add(psH[:, :], psH[:, :], b2bc[:, :])
                mtg = ctx["mt"][:, i0 : i0 + G].unsqueeze(2).broadcast_to([128, G, D])
                ht = htpool.tile([128, GF], bf16)
                ht3 = ht[:, :].rearrange("k (i d) -> k i d", i=G)
                psH3 = psH[:, :].rearrange("k (i d) -> k i d", i=G)
                use_dve = relu2_mode == "dve" or (
                    relu2_mode == "alt" and (ctx["b"] * ng + g) % 16 in dve_pat
                )
                if use_dve:
                    # ht = relu(psH) * m  ==  (psH max 0) * m, one fused DVE op
                    nc.vector.scalar_tensor_tensor(
                        ht3, psH3, 0.0, mtg, Alu.max, Alu.mult
                    )
                else:
                    ht0 = ht0pool.tile([128, GF], bf16, tag="ht0")
                    nc.scalar.activation(ht0[:, :], psH[:, :], Relu)
                    mtgb = (
                        ctx["mtb"][:, i0 : i0 + G]
                        .unsqueeze(2)
                        .broadcast_to([128, G, D])
                    )
                    nc.gpsimd.tensor_mul(
                        ht3, ht0[:, :].rearrange("k (i d) -> k i d", i=G), mtgb
                    )
                ctx["ht"] = ht

            def S2(i):
                ctx = ctxs[i]
                b, g = ctx["b"], ctx["g"]
                i0 = g * G
                psO = psOpool.tile([128, GF], f32)
                nc.tensor.matmul(
                    psO[:, :], ctx["at"][:, :], ctx["ht"][:, :],
                    start=True, stop=True,
                )
                psO3 = psO[:, :].rearrange("j (i d) -> j i d", i=G)
                mtg = ctx["mt"][:, i0 : i0 + G].unsqueeze(2).broadcast_to([128, G, D])
                if bias_mode:
                    tmpc = tmpc_pool.tile([128, GF], f32)
                    ubg = (
                        ctx["ub"][:, i0 : i0 + G]
                        .unsqueeze(2)
                        .broadcast_to([128, G, D])
                    )
                    nc.vector.tensor_mul(
                        tmpc[:, :].rearrange("j (i d) -> j i d", i=G),
                        ubg,
                        cb[:, :].rearrange("j (i d) -> j i d", i=G),
                    )
                    nc.vector.tensor_add(psO[:, :], psO[:, :], tmpc[:, :])
                ot3 = (
                    ctx["obuf"][:, g * GF : (g + 1) * GF]
                    .rearrange("j (i d) -> j i d", i=G)
                )
                nc.vector.tensor_mul(ot3, psO3, mtg)
                # last batch stores in finer chunks so the final DMA drains fast
                oc = ocst if b < bsh - 1 else max(1, ocst // 2)
                if (g + 1) % oc == 0:
                    c0 = (g - g % oc) * GF
                    c1 = (g + 1) * GF
                    # output stores can ride the SWDGE ring (gpsimd) so they
                    # never queue ahead of input prefetches on the HWDGE ring
                    oeng = nc.gpsimd if cfg.get("oswdge") else nc.sync
                    oeng.dma_start(O_d[b][:, c0:c1], ctx["obuf"][:, c0:c1])

            for i in range(total):
                if i % sgn == 0:
                    S0(i)
                if i >= 1:
                    S1(i - 1)
                if i >= 2:
                    S2(i - 2)
            S1(total - 1)
            S2(total - 2)
            S2(total - 1)

            if bias_mode:
                ub_pool.__exit__(None, None, None)
                tmpc_pool.__exit__(None, None, None)
                psU_pool.__exit__(None, None, None)

    nc.compile()
    nc.m = get_hw_module(nc.m)
    return nc


def kernel(X, A, mask, W1, b1, W2, b2):
    import ml_dtypes
    from concourse.bass_utils import run_bass_kernel_spmd

    bf = ml_dtypes.bfloat16
    cfg = dict(
        relu2="alt", dve_pat=(1, 3, 5, 9, 11, 13), sgn=2, ochunk=4, ichunk=4
    )
    import json
    import os

    if os.environ.get("KCFG"):
        _o = json.loads(os.environ["KCFG"])
        if "dve_pat" in _o:
            _o["dve_pat"] = tuple(_o["dve_pat"])
        cfg.update(_o)

    X = np.asarray(X, dtype=np.float32)
    A = np.asarray(A, dtype=np.float32)
    mask = np.asarray(mask)
    W1 = np.asarray(W1, dtype=np.float32)
    W2 = np.asarray(W2, dtype=np.float32)
    b1 = np.asarray(b1, dtype=np.float32)
    b2 = np.asarray(b2, dtype=np.float32)

    bias_mode = bool(np.any(b1 != 0.0) or np.any(b2 != 0.0))
    key = (bias_mode, tuple(sorted(cfg.items())))
    if key not in _built:
        _built[key] = _build(bias_mode, cfg)
    nc = _built[key]

    # XT[b, (half,d), (g,p,k)] for i = 8g + 2p + half
    XT = np.ascontiguousarray(
        X.reshape(B, NG, NP, 2, N, D).transpose(0, 3, 5, 1, 2, 4)
    ).reshape(B, 128, NG * GF).astype(bf)
    Ab = A.astype(bf)
    MTf = np.ascontiguousarray(np.swapaxes(mask, 1, 2)).astype(np.float32)
    MTb = MTf.astype(bf)

    w1q = np.zeros((128, 128), dtype=np.float32)
    w1q[0:64, 0:64] = W1
    w1q[64:128, 64:128] = W1
    w2q = np.zeros((128, 128), dtype=np.float32)
    w2q[0:64, 0:64] = W2
    w2q[64:128, 64:128] = W2
    b1d = np.concatenate([b1, b1], axis=0).reshape(128, 1).astype(np.float32)

    shared = {
        "W1Q": w1q.astype(bf),
        "W2Q": w2q.astype(bf),
        "B1D": b1d,
    }
    if bias_mode:
        c = np.maximum(np.maximum(b1, 0.0) @ W2 + b2, 0.0).astype(np.float32)
        shared["B2BC"] = np.tile(b2, (128, G)).astype(np.float32)
        shared["CB"] = np.tile(c, (128, G)).astype(np.float32)

    in_maps = []
    for cid in range(NC):
        sl = slice(cid * BSH, (cid + 1) * BSH)
        in_maps.append(
            {"XT": XT[sl], "A": Ab[sl], "MT": MTf[sl], "MTB": MTb[sl], **shared}
        )
    global _last_in_maps
    _last_in_maps = in_maps

    try:
        res = run_bass_kernel_spmd(nc, in_maps, core_ids=list(range(NC)))
    except Exception:
        res = run_bass_kernel_spmd(nc, in_maps, core_ids=list(range(NC)))
    OT = np.concatenate([res.results[c]["OUT"] for c in range(NC)], axis=0)
    # OT[b, j, (g, ig, d)] -> out[b, i=8g+ig, j, d]
    out = (
        OT.astype(np.float32)
        .reshape(B, N, NG, G, D)
        .transpose(0, 2, 3, 1, 4)
        .reshape(B, N, N, D)
    )
    return np.ascontiguousarray(out)


# revision 32
# speedup vs baseline: 1.0896x; 1.0154x over previous
"""Trainium2 Bass kernel for nn_NestedConv (gnn_message_passing).

Math (per b, i):
    Xm       = X[b,i] * mask[b,i,:,None]                 # (N,D), rows k masked
    h1       = relu(Xm @ W1 + b1)                        # (N,D)
    h        = relu(h1 @ W2 + b2)                        # (N,D)
    out[b,i] = (A[b].T @ h) * mask[b,i,:,None]           # (N,D), rows j masked

Key restructuring vs the obvious dataflow:
  - X is uploaded host-pre-transposed+bf16 as XT[b, (half,d), (g,p,k)] so the
    MLP contraction dim d sits on partitions with no on-chip transposes.
  - The input row-mask is deferred: rowwise MLP maps 0-rows to 0-rows when
    b1=b2=0, so masking h (natural layout) == masking X. For nonzero biases
    the exact correction  out += (A^T (1-m_i)) c^T  with c = MLP(0-row) is
    added per batch (one extra matmul) + per group (two DVE ops).
  - All matmuls bf16 (A and mask are 0/1-exact in bf16):
      mm1: stationary blockdiag[W1,W1], moving XT slice      -> h1^T pairs
      mm2: stationary h1^T pair, moving blockdiag[W2,W2]     -> h natural
      mm3: stationary A[b], moving h (8 i's batched)         -> out rows j
  - Work is grouped in super-groups of 16 root nodes (1024-wide tiles) to
    amortize fixed op costs; mm2's psH and mm3's psO reuse the same 2-bank
    PSUM tile sequentially so everything fits in 8 banks double-buffered.
  - Elementwise work is split across ACT/DVE/GPSIMD:
      relu1 on ACT; relu2+hmask alternates (even sg: ACT relu + GPSIMD
      mask-mul; odd sg: one fused DVE (max,mult) op); outmask on DVE.
  - Input loaded in 512 KiB chunks (compute starts ~2 us in), output stored
    bf16 from a per-batch SBUF buffer in 1 MiB chunks; host re-transposes.

Sharding: data-parallel over batch dim B=64 across 8 NeuronCores (8 b's each).
"""

import sys

sys.path.insert(0, "/opt/trn_rl_repo")

import numpy as np

B, N, D = 64, 128, 64
NC = 8
BSH = B // NC  # batches per core
G = 8  # root nodes i per group
NG = N // G  # groups per batch
GF = G * D  # free size of one group: 512
NP = G // 2  # stationary pairs per group: 4

_built = {}
_last_in_maps = None


def _build(bias_mode: bool, cfg: dict, bsh: int = BSH, ng: int = NG):
    import concourse.bacc as bacc
    import concourse.mybir as mybir
    from concourse import tile
    from concourse.bass_interp import get_hw_module

    f32 = mybir.dt.float32
    bf16 = mybir.dt.bfloat16
    Relu = mybir.ActivationFunctionType.Relu
    Alu = mybir.AluOpType

    relu2_mode = cfg.get("relu2", "alt")  # alt | dve | act_gps
    # global group indices (mod 16) that use the fused-DVE relu2 path;
    # 7/16 balances ACT (relu1 + the other 9/16 relu2) against DVE
    # (outmask + these). The last batch goes 1/2 so the slow GPSIMD
    # mask-mul stays off the drain critical path.
    dve_pat = cfg.get("dve_pat", (1, 3, 5, 7, 9, 11, 13))
    sgn = 1 if bias_mode else cfg.get("sgn", 2)  # groups sharing one mm1/relu1
    ocst = cfg.get("ochunk", 4)  # groups per output-store chunk
    icn = cfg.get("ichunk", 4)  # input chunks per batch

    nc = bacc.Bacc("TRN2", target_bir_lowering=False, debug=False, num_devices=1)

    XT_d = nc.dram_tensor("XT", [bsh, 128, ng * GF], bf16, kind="ExternalInput").ap()
    A_d = nc.dram_tensor("A", [bsh, 128, 128], bf16, kind="ExternalInput").ap()
    MT_d = nc.dram_tensor("MT", [bsh, 128, 128], f32, kind="ExternalInput").ap()
    MTB_d = nc.dram_tensor("MTB", [bsh, 128, 128], bf16, kind="ExternalInput").ap()
    W1_d = nc.dram_tensor("W1Q", [128, 128], bf16, kind="ExternalInput").ap()
    W2_d = nc.dram_tensor("W2Q", [128, 128], bf16, kind="ExternalInput").ap()
    B1_d = nc.dram_tensor("B1D", [128, 1], f32, kind="ExternalInput").ap()
    if bias_mode:
        B2_d = nc.dram_tensor("B2BC", [128, GF], f32, kind="ExternalInput").ap()
        CB_d = nc.dram_tensor("CB", [128, GF], f32, kind="ExternalInput").ap()
    O_d = nc.dram_tensor("OUT", [bsh, 128, ng * GF], bf16, kind="ExternalOutput").ap()

    with tile.TileContext(nc) as tc:
        with (
            tc.tile_pool(name="const", bufs=1) as cpool,
            tc.tile_pool(name="xb", bufs=2) as xbpool,
            tc.tile_pool(name="bmeta", bufs=2) as bmpool,
            tc.tile_pool(name="ob", bufs=2) as obpool,
            tc.tile_pool(name="h1", bufs=3) as h1pool,
            tc.tile_pool(name="ht", bufs=3) as htpool,
            tc.tile_pool(name="ht0", bufs=2) as ht0pool,
            tc.tile_pool(name="psH1", bufs=2, space="PSUM") as psH1pool,
            tc.tile_pool(name="psH", bufs=2, space="PSUM") as psHpool,
            tc.tile_pool(name="psO", bufs=2, space="PSUM") as psOpool,
        ):
            ub_pool = tmpc_pool = psU_pool = None
            if bias_mode:
                ub_pool = tc.tile_pool(name="ub", bufs=2).__enter__()
                tmpc_pool = tc.tile_pool(name="tmpc", bufs=2).__enter__()
                psU_pool = tc.tile_pool(name="psU", bufs=1, space="PSUM").__enter__()

            # First compute needs xbT chunk0 + w1q; issue chunk0 first (each
            # dma_start costs ~650ns of SP-queue serial issue time).
            xbT0 = xbpool.tile([128, ng * GF], bf16)
            nc.sync.dma_start(xbT0[:, 0:GF], XT_d[0][:, 0:GF])
            w1q = cpool.tile([128, 128], bf16, tag="w1q")
            nc.sync.dma_start(w1q[:, :], W1_d)
            w2q = cpool.tile([128, 128], bf16, tag="w2q")
            nc.sync.dma_start(w2q[:, :], W2_d)
            b1d = cpool.tile([128, 1], f32, tag="b1d")
            nc.sync.dma_start(b1d[:, :], B1_d)
            # Warm the ACT function table during the first input DMA so the
            # one-time ACT_TABLE_LOAD (~1.3us) is off the critical path.
            warm = cpool.tile([128, 1], f32, tag="warm")
            nc.scalar.activation(warm[:, :], b1d[:, :], Relu)
            if bias_mode:
                b2bc = cpool.tile([128, GF], f32, tag="b2bc")
                nc.sync.dma_start(b2bc[:, :], B2_d)
                cb = cpool.tile([128, GF], f32, tag="cb")
                nc.sync.dma_start(cb[:, :], CB_d)

            batch_tiles = {}
            FW = ng * GF  # full batch free width: 8192

            def load_batch(b):
                if b >= bsh:
                    return
                cw = FW // icn
                if b == 0:
                    # chunk0 (one group) was issued before the consts
                    xbT = xbT0
                    cuts = [GF, cw] + [c * cw for c in range(2, icn + 1)]
                else:
                    xbT = xbpool.tile([128, FW], bf16)
                    cuts = [c * cw for c in range(icn + 1)]
                nc.sync.dma_start(
                    xbT[:, cuts[0] : cuts[1]], XT_d[b][:, cuts[0] : cuts[1]]
                )
                at = bmpool.tile([128, 128], bf16, tag="at")
                nc.sync.dma_start(at[:, :], A_d[b])
                mt = bmpool.tile([128, 128], f32, tag="mt")
                nc.sync.dma_start(mt[:, :], MT_d[b])
                mtb = bmpool.tile([128, 128], bf16, tag="mtb")
                nc.sync.dma_start(mtb[:, :], MTB_d[b])
                for c0, c1 in zip(cuts[1:], cuts[2:]):
                    nc.sync.dma_start(xbT[:, c0:c1], XT_d[b][:, c0:c1])
                batch_tiles[b] = dict(xbT=xbT, at=at, mt=mt, mtb=mtb)

            load_batch(0)

            total = bsh * ng
            ctxs = [None] * total

            def S0(i):
                # one S0 covers a super-group of sgn consecutive groups:
                # sgn N=512 matmuls into one PSUM tile, a single wide
                # relu1, shared h1t tile.
                b, g = divmod(i, ng)
                if g == ng // 4:
                    load_batch(b + 1)
                t = batch_tiles[b]
                if g == 0:
                    if bias_mode:
                        # U[j,i] = sum_k A[k,j] (1 - m[k,i]); c-column correction
                        omtb = ht0pool.tile([128, 128], bf16, tag="omtb")
                        nc.vector.tensor_scalar(
                            omtb[:, :], t["mt"][:, :], 1.0, -1.0,
                            Alu.subtract, Alu.mult,
                        )
                        psU = psU_pool.tile([128, 128], f32)
                        nc.tensor.matmul(
                            psU[:, :], t["at"][:, :], omtb[:, :],
                            start=True, stop=True,
                        )
                        ub = ub_pool.tile([128, 128], f32)
                        nc.scalar.copy(ub[:, :], psU[:, :])
                        batch_tiles[b]["ub"] = ub
                    obuf = obpool.tile([128, FW], bf16)
                    batch_tiles[b]["obuf"] = obuf

                psH1 = psH1pool.tile([128, sgn * GF], f32)
                for s in range(sgn):
                    gs = g + s
                    nc.tensor.matmul(
                        psH1[:, s * GF : (s + 1) * GF],
                        w1q[:, :],
                        t["xbT"][:, gs * GF : (gs + 1) * GF],
                        start=True, stop=True,
                    )
                h1t = h1pool.tile([128, sgn * GF], bf16)
                nc.scalar.activation(h1t[:, :], psH1[:, :], Relu, bias=b1d[:, 0:1])
                for s in range(sgn):
                    ctx = dict(b=b, g=g + s, **t)
                    ctx["obuf"] = batch_tiles[b]["obuf"]
                    if bias_mode:
                        ctx["ub"] = batch_tiles[b]["ub"]
                    ctx["h1t"] = h1t
                    ctx["h1off"] = s * GF
                    ctxs[i + s] = ctx

            def S1(i):
                ctx = ctxs[i]
                g = ctx["g"]
                i0 = g * G
                h1t = ctx["h1t"]
                off = ctx["h1off"]
                psH = psHpool.tile([128, GF], f32)
                for p in range(NP):
                    nc.tensor.matmul(
                        psH[:, p * 128 : (p + 1) * 128],
                        h1t[:, off + p * 128 : off + (p + 1) * 128],
                        w2q[:, :],
                        start=True, stop=True,
                    )
                if bias_mode:
                    nc.vector.tensor_add(psH[:, :], psH[:, :], b2bc[:, :])
                mtg = ctx["mt"][:, i0 : i0 + G].unsqueeze(2).broadcast_to([128, G, D])
                ht = htpool.tile([128, GF], bf16)
                ht3 = ht[:, :].rearrange("k (i d) -> k i d", i=G)
                psH3 = psH[:, :].rearrange("k (i d) -> k i d", i=G)
                use_dve = relu2_mode == "dve" or (
                    relu2_mode == "alt" and (ctx["b"] * ng + g) % 16 in dve_pat
                )
                if use_dve:
                    # ht = relu(psH) * m  ==  (psH max 0) * m, one fused DVE op
                    nc.vector.scalar_tensor_tensor(
                        ht3, psH3, 0.0, mtg, Alu.max, Alu.mult
                    )
                else:
                    ht0 = ht0pool.tile([128, GF], bf16, tag="ht0")
                    nc.scalar.activation(ht0[:, :], psH[:, :], Relu)
                    mtgb = (
                        ctx["mtb"][:, i0 : i0 + G]
                        .unsqueeze(2)
                        .broadcast_to([128, G, D])
                    )
                    nc.gpsimd.tensor_mul(
                        ht3, ht0[:, :].rearrange("k (i d) -> k i d", i=G), mtgb
                    )
                ctx["ht"] = ht

            def S2(i):
                ctx = ctxs[i]
                b, g = ctx["b"], ctx["g"]
                i0 = g * G
                psO = psOpool.tile([128, GF], f32)
                nc.tensor.matmul(
                    psO[:, :], ctx["at"][:, :], ctx["ht"][:, :],
                    start=True, stop=True,
                )
                psO3 = psO[:, :].rearrange("j (i d) -> j i d", i=G)
                mtg = ctx["mt"][:, i0 : i0 + G].unsqueeze(2).broadcast_to([128, G, D])
                if bias_mode:
                    tmpc = tmpc_pool.tile([128, GF], f32)
                    ubg = (
                        ctx["ub"][:, i0 : i0 + G]
                        .unsqueeze(2)
                        .broadcast_to([128, G, D])
                    )
                    nc.vector.tensor_mul(
                        tmpc[:, :].rearrange("j (i d) -> j i d", i=G),
                        ubg,
                        cb[:, :].rearrange("j (i d) -> j i d", i=G),
                    )
                    nc.vector.tensor_add(psO[:, :], psO[:, :], tmpc[:, :])
                ot3 = (
                    ctx["obuf"][:, g * GF : (g + 1) * GF]
                    .rearrange("j (i d) -> j i d", i=G)
                )
                nc.vector.tensor_mul(ot3, psO3, mtg)
                # last batch stores in finer chunks so the final DMA drains fast
                oc = ocst
                if b == bsh - 1:
                    oc = max(1, ocst // 2) if g < ng - 4 else 1
                if (g + 1) % oc == 0:
                    c0 = (g - g % oc) * GF
                    c1 = (g + 1) * GF
                    # output stores can ride the SWDGE ring (gpsimd) so they
                    # never queue ahead of input prefetches on the HWDGE ring
                    oeng = nc.gpsimd if cfg.get("oswdge") else nc.sync
                    oeng.dma_start(O_d[b][:, c0:c1], ctx["obuf"][:, c0:c1])

            for i in range(total):
                if i % sgn == 0:
                    S0(i)
                if i >= 1:
                    S1(i - 1)
                if i >= 2:
                    S2(i - 2)
            S1(total - 1)
            S2(total - 2)
            S2(total - 1)

            if bias_mode:
                ub_pool.__exit__(None, None, None)
                tmpc_pool.__exit__(None, None, None)
                psU_pool.__exit__(None, None, None)

    nc.compile()
    nc.m = get_hw_module(nc.m)
    return nc


def kernel(X, A, mask, W1, b1, W2, b2):
    import ml_dtypes
    from concourse.bass_utils import run_bass_kernel_spmd

    bf = ml_dtypes.bfloat16
    cfg = dict(
        relu2="alt", dve_pat=(1, 3, 5, 9, 11, 13), sgn=2, ochunk=4, ichunk=4
    )
    import json
    import os

    if os.environ.get("KCFG"):
        _o = json.loads(os.environ["KCFG"])
        if "dve_pat" in _o:
            _o["dve_pat"] = tuple(_o["dve_pat"])
        cfg.update(_o)

    X = np.asarray(X, dtype=np.float32)
    A = np.asarray(A, dtype=np.float32)
    mask = np.asarray(mask)
    W1 = np.asarray(W1, dtype=np.float32)
    W2 = np.asarray(W2, dtype=np.float32)
    b1 = np.asarray(b1, dtype=np.float32)
    b2 = np.asarray(b2, dtype=np.float32)

    bias_mode = bool(np.any(b1 != 0.0) or np.any(b2 != 0.0))
    key = (bias_mode, tuple(sorted(cfg.items())))
    if key not in _built:
        _built[key] = _build(bias_mode, cfg)
    nc = _built[key]

    # XT[b, (half,d), (g,p,k)] for i = 8g + 2p + half
    XT = np.ascontiguousarray(
        X.reshape(B, NG, NP, 2, N, D).transpose(0, 3, 5, 1, 2, 4)
    ).reshape(B, 128, NG * GF).astype(bf)
    Ab = A.astype(bf)
    MTf = np.ascontiguousarray(np.swapaxes(mask, 1, 2)).astype(np.float32)
    MTb = MTf.astype(bf)

    w1q = np.zeros((128, 128), dtype=np.float32)
    w1q[0:64, 0:64] = W1
    w1q[64:128, 64:128] = W1
    w2q = np.zeros((128, 128), dtype=np.float32)
    w2q[0:64, 0:64] = W2
    w2q[64:128, 64:128] = W2
    b1d = np.concatenate([b1, b1], axis=0).reshape(128, 1).astype(np.float32)

    shared = {
        "W1Q": w1q.astype(bf),
        "W2Q": w2q.astype(bf),
        "B1D": b1d,
    }
    if bias_mode:
        c = np.maximum(np.maximum(b1, 0.0) @ W2 + b2, 0.0).astype(np.float32)
        shared["B2BC"] = np.tile(b2, (128, G)).astype(np.float32)
        shared["CB"] = np.tile(c, (128, G)).astype(np.float32)

    in_maps = []
    for cid in range(NC):
        sl = slice(cid * BSH, (cid + 1) * BSH)
        in_maps.append(
            {"XT": XT[sl], "A": Ab[sl], "MT": MTf[sl], "MTB": MTb[sl], **shared}
        )
    global _last_in_maps
    _last_in_maps = in_maps

    try:
        res = run_bass_kernel_spmd(nc, in_maps, core_ids=list(range(NC)))
    except Exception:
        res = run_bass_kernel_spmd(nc, in_maps, core_ids=list(range(NC)))
    OT = np.concatenate([res.results[c]["OUT"] for c in range(NC)], axis=0)
    # OT[b, j, (g, ig, d)] -> out[b, i=8g+ig, j, d]
    out = (
        OT.astype(np.float32)
        .reshape(B, N, NG, G, D)
        .transpose(0, 2, 3, 1, 4)
        .reshape(B, N, N, D)
    )
    return np.ascontiguousarray(out)
